# revision 6
# baseline (speedup 1.0000x reference)
"""Trainium2 Bass kernel for nn_DCTModel: bilinear x8 upsample + RGB->YCbCr +
8x8 block DCT + channel selection, fused into two dense matmuls per plane.

Math: the whole reference pipeline is linear in x (all affine offsets only
shift the DC coefficient, which is excluded from the output), so

    out[b, r, (u,i), (v,j)] = (Th @ Xhat[b,r] @ Tw^T)[(u,i), (v,j)]

with Xhat[b,r] = sum_c 127.5*RGB2YCBCR[r,c] * x[b,c]  (112x112),
Th = C @ Ah (DCT-harmonics x bilinear-upsample, [8*112, 112]) with the
orthonormal alpha(u)/2 scale folded in; Tw identical. 54 of the 64 (u,v)
DCT channels are kept.

Output-DMA layout: TRN2 SDMA writes below 512B per descriptor do
read-modify-write at HBM (~half bandwidth). A naive [i-part, (c, j)] staging
tile emits 448B descriptors (one 112-f32 output row). Instead, the
stationary operand of matmul 2 (columns of Th per u-block, baked into the
tht constant) is permuted so PSUM partitions 0-55 hold even output rows
i=2a and partitions 64-119 hold odd rows i=2a+1. The PSUM->SBUF copies
then build a row-pair staging tile: partition a holds output rows
(2a, 2a+1) back to back, so every output descriptor covers two
DRAM-adjacent rows = 896B >= 512B -> line rate.

Measured per-descriptor SDMA cost is ~(bytes/22.5GB/s + 14ns) on each of
16 engines, so 896B descriptors lift the output-DMA ceiling from ~230 to
~265 GB/s. Each output fill gets its own staging tile (double-buffered):
a single per-plane staging tile would serialize fill N's copies behind
fill N-1's DMA (whole-tile write-after-read). Fill DMAs alternate
between the SP HWDGE ring and the (otherwise idle) GPSIMD SWDGE ring so
neither sequencer backs up.

Per (b, r) plane:
  mix (DVE)           Xhat = sum_c M2[r,c] x[b,c]            -> fp16 [112,112]
  mm1 (PE, fp16)      a1t[w,(u,i)] = Xhat^T @ ThT(perm)      -> PSUM -> fp16
  mm2 (PE, fp16)      per u-slice: a1t_u^T @ TwT             -> PSUM f32
                      (banked 4 channels per 512-f32 PSUM bank)
  copies (DVE+ACT)    even rows ps[0:56], odd rows ps[64:120] -> pair staging
  DMA (sync HWDGE)    staging fill -> out[b, ch-slice], 896B descriptors

PSUM: ring of 2 x [128, 2048] (4-bank) tiles; fills per plane:
F0 = mm1 (2x512), F1-F3 = 16 output channels each ([4,4,4,4] banks),
F4 = 6 channels ([2,2,2] banks).

Sharding: pure data parallel, batch 16 -> 2 per core across 8 cores.
"""

import numpy as np

L = 112
SIZE = 8
BS_PER_CORE = 2
N_CORES = 8
SUB_CHANNELS = {0, 1, 2, 3, 4, 5, 8, 9, 16, 24}

RGB2YCBCR = np.asarray(
    [[0.299, 0.587, 0.114],
     [-0.168736, -0.331264, 0.5],
     [0.5, -0.418688, -0.081312]], np.float32)

# per-u: first selected v (selected v's are the contiguous range [V_LO[u], 8))
V_LO = []
M_START = []
_m = 0
for _u in range(SIZE):
    _sel = [_v for _v in range(SIZE) if _u * SIZE + _v not in SUB_CHANNELS]
    assert _sel == list(range(_sel[0], SIZE))
    V_LO.append(_sel[0])
    M_START.append(_m)
    _m += len(_sel)
assert _m == 54
M_OF_U = [M_START[u + 1] if u + 1 < SIZE else 54 for u in range(SIZE)]

# channel m -> (u, v)
U_OF_M = np.zeros(54, np.int64)
V_OF_M = np.zeros(54, np.int64)
for _u in range(SIZE):
    for _mm in range(M_START[_u], M_OF_U[_u]):
        U_OF_M[_mm] = _u
        V_OF_M[_mm] = V_LO[_u] + (_mm - M_START[_u])

# output fills: (ch_start, n_ch_per_bank, n_banks)
FILLS = [(0, 4, 4), (16, 4, 4), (32, 4, 4), (48, 2, 3)]

# matmul cut list per fill: (bank_idx, in_bank_ch_off, u, m_lo, n_ch)
def _mm2_cuts():
    cuts = []
    for fs, nper, nb in FILLS:
        mms = []
        for bi in range(nb):
            lo = fs + bi * nper
            hi = lo + nper
            m = lo
            while m < hi:
                u = int(U_OF_M[m])
                e = min(hi, M_OF_U[u])
                mms.append((bi, m - lo, u, m, e - m))
                m = e
        cuts.append(mms)
    return cuts

MM2_CUTS = _mm2_cuts()


def _build_consts():
    """tht[h, u*128+k]: permuted so mm2's PSUM partition k holds output row
    i=2k (k<56) / i=2(k-64)+1 (64<=k<120); twt[w, v*112+j] = Tw[v,j,w]."""
    Lo = L * SIZE
    src = np.arange(Lo) * (L - 1) / (Lo - 1)
    i0 = np.minimum(np.floor(src).astype(np.int64), L - 2)
    w = (src - i0).astype(np.float32)
    A = np.zeros((Lo, L), np.float32)
    A[np.arange(Lo), i0] = 1.0 - w
    A[np.arange(Lo), i0 + 1] = w

    xg = np.arange(SIZE) + 0.5
    ug = np.arange(SIZE)
    h = np.cos(np.outer(xg, ug) * np.pi / SIZE).astype(np.float32)
    alpha = np.ones(SIZE, np.float32)
    alpha[0] = 1.0 / np.sqrt(2.0)

    Ab = A.reshape(L, SIZE, L)  # [i, x, h']
    Th = np.einsum('xu,ixh->uih', h, Ab).astype(np.float32)
    Th = Th * (alpha / 2.0)[:, None, None]  # [u, i, h]

    ThTp = np.zeros((L, SIZE * 128), np.float16)
    for u in range(SIZE):
        for k in range(56):
            ThTp[:, u * 128 + k] = Th[u, 2 * k, :]
            ThTp[:, u * 128 + 64 + k] = Th[u, 2 * k + 1, :]
    TwT = np.ascontiguousarray(
        Th.transpose(2, 0, 1).reshape(L, SIZE * L)).astype(np.float16)
    return ThTp, TwT


_CACHE = {}


def _build_program():
    import concourse.bacc as bacc
    import concourse.mybir as mybir
    import concourse.tile as tile

    f32 = mybir.dt.float32
    f16 = mybir.dt.float16
    mult = mybir.AluOpType.mult
    add = mybir.AluOpType.add

    M2 = (127.5 * RGB2YCBCR).astype(np.float32)

    nc = bacc.Bacc(
        "TRN2",
        target_bir_lowering=False,
        debug=False,
        enable_asserts=False,
        num_devices=N_CORES,
    )
    x_d = nc.dram_tensor("x", [BS_PER_CORE, 3, L, L], f32, kind="ExternalInput").ap()
    tht_d = nc.dram_tensor("tht", [L, SIZE * 128], f16, kind="ExternalInput").ap()
    twt_d = nc.dram_tensor("twt", [L, SIZE * L], f16, kind="ExternalInput").ap()
    out_d = nc.dram_tensor(
        "out", [BS_PER_CORE, 162, L, L], f32, kind="ExternalOutput"
    ).ap()

    with tile.TileContext(nc) as tc:
        with tc.tile_pool(name="consts", bufs=1) as cpool, \
             tc.tile_pool(name="xin", bufs=2) as xpool, \
             tc.tile_pool(name="mix", bufs=2) as mpool, \
             tc.tile_pool(name="a1", bufs=2) as apool, \
             tc.tile_pool(name="stg", bufs=2) as spool, \
             tc.tile_pool(name="ring", bufs=2, space="PSUM") as ppool:
            xbs = []
            for b in range(BS_PER_CORE):
                xb = xpool.tile([L, 3, L], f32, name=f"xb{b}", tag="xb")
                if b == 0:
                    nc.sync.dma_start(xb[:], x_d[b].transpose([1, 0, 2]))
                xbs.append(xb)
            tht = cpool.tile([L, SIZE * 128], f16, name="tht_sb")
            nc.sync.dma_start(tht[:], tht_d[:])
            twt = cpool.tile([L, SIZE * L], f16, name="twt_sb")
            nc.sync.dma_start(twt[:], twt_d[:])
            nc.sync.dma_start(xbs[1][:], x_d[1].transpose([1, 0, 2]))

            # balance PSUM->SBUF copies between DVE and ACT by modeled cost
            eng_cost = {"v": 0.0, "s": 0.0}

            def psum_copy(dst, src, ncols):
                cv = ncols * 1.042 + 320
                cs = ncols * 0.833 + 242
                if eng_cost["v"] + cv <= eng_cost["s"] + cs:
                    nc.vector.tensor_copy(dst, src)
                    eng_cost["v"] += cv
                else:
                    nc.scalar.copy(dst, src)
                    eng_cost["s"] += cs

            def emit_mix(k, b, r):
                xb = xbs[b]
                tmp = mpool.tile([L, L], f32, name=f"tmp{k}", tag="tmp")
                xh = mpool.tile([L, L], f16, name=f"xh{k}", tag="xh")
                nc.vector.tensor_scalar_mul(tmp[:], xb[:, 1, :], float(M2[r, 1]))
                nc.vector.scalar_tensor_tensor(
                    tmp[:], xb[:, 0, :], float(M2[r, 0]), tmp[:], mult, add)
                nc.vector.scalar_tensor_tensor(
                    xh[:], xb[:, 2, :], float(M2[r, 2]), tmp[:], mult, add)
                # mix cost lands on DVE
                eng_cost["v"] += 3 * (112 * 1.042 + 180)
                return xh

            planes = [(b, r) for b in range(BS_PER_CORE) for r in range(3)]
            xh_k = emit_mix(0, *planes[0])

            for k, (b, r) in enumerate(planes):
                xh = xh_k
                # F0: mm1 -> ring tile banks 0-1, cast-copy to a1t fp16
                t0 = ppool.tile([128, 2048], f32, name=f"f0_{k}", tag="ring")
                for half in range(2):
                    nc.tensor.matmul(
                        t0[0:L, 512 * half:512 * (half + 1)],
                        lhsT=xh[:],
                        rhs=tht[:, 512 * half:512 * (half + 1)],
                        start=True, stop=True)
                a1t = apool.tile([L, SIZE * 128], f16, name=f"a1t{k}", tag="a1t")
                psum_copy(
                    a1t[:].rearrange("p (x n) -> p x n", x=2),
                    t0[0:L, 0:1024].rearrange("p (x n) -> p x n", x=2),
                    1024)

                # next plane's mix goes ahead of this plane's copy flood
                if k + 1 < len(planes):
                    xh_k = emit_mix(k + 1, *planes[k + 1])

                c0 = r * 54
                for fi, (fs, nper, nb) in enumerate(FILLS):
                    tf = ppool.tile([128, 2048], f32, name=f"f{fi + 1}_{k}",
                                    tag="ring")
                    for (bi, boff, u, mlo, nch) in MM2_CUTS[fi]:
                        v = V_LO[u] + (mlo - M_START[u])
                        nc.tensor.matmul(
                            tf[:, bi * 512 + boff * L:
                               bi * 512 + (boff + nch) * L],
                            lhsT=a1t[:, u * 128:(u + 1) * 128],
                            rhs=twt[:, v * L:(v + nch) * L],
                            start=True, stop=True)
                    nf = nper * nb
                    src = tf[:, 0:nb * 512].rearrange(
                        "p (x q) -> p x q", x=nb)[:, :, 0:nper * L].rearrange(
                        "p x (y j) -> p x y j", y=nper)
                    stage = spool.tile([56, nf, 2, L], f32,
                                       name=f"st{k}_{fi}", tag=f"stg{fi}")
                    psum_copy(stage[:, :, 0, :], src[0:56], nf * L)
                    psum_copy(stage[:, :, 1, :], src[64:120], nf * L)
                    dma_eng = nc.sync if fi % 2 == 0 else nc.gpsimd
                    dma_eng.dma_start(
                        out_d[b, c0 + fs:c0 + fs + nf].rearrange(
                            "c (a h) j -> a c (h j)", a=56),
                        stage[:].rearrange("p c h j -> p c (h j)"))

    nc.compile()
    return nc


def kernel(x: np.ndarray) -> np.ndarray:
    from concourse import bass_utils

    x = np.ascontiguousarray(np.asarray(x, np.float32))
    assert x.shape == (BS_PER_CORE * N_CORES, 3, L, L)

    if "nc" not in _CACHE:
        _CACHE["nc"] = _build_program()
        _CACHE["consts"] = _build_consts()
    nc = _CACHE["nc"]
    ThT, TwT = _CACHE["consts"]

    in_maps = [
        {"x": x[c * BS_PER_CORE:(c + 1) * BS_PER_CORE], "tht": ThT, "twt": TwT}
        for c in range(N_CORES)
    ]
    res = bass_utils.run_bass_kernel_spmd(nc, in_maps, core_ids=list(range(N_CORES)))
    out = np.concatenate([res.results[c]["out"] for c in range(N_CORES)], axis=0)
    return out
